# revision 6
# baseline (speedup 1.0000x reference)
"""DAAGCN Trainium2 kernel — node-sharded across 8 NeuronCores.

Strategy: shard the 1024 graph nodes over 8 cores (128 each), full batch
B=64 per core. The support matrices S_t, Chebyshev S2_t and the per-node
weight matrices depend only on model parameters (node/time embeddings),
so they are constant-folded on the host. Each GRU step on device:
AllGather(state) -> graph-conv matmuls (S^T stationary) -> PE transposes
to feat-major -> per-node matmuls (bias folded as an extra contraction
row) -> sigmoid -> AllGather(z*state) -> update branch -> tanh -> GRU
state update. Final LayerNorm+end_conv are folded into one matmul with
column-statistics rows.

Layout conventions (per core):
  feat-major: [feat, b*128 + n_local]   (free size FB = 8192)
  node-major: [n, b*64 + h]             (free size FH = 4096)
"""
import sys

sys.path.insert(0, "/opt/trn_rl_repo")

import numpy as np
import ml_dtypes

import concourse.bass as bass
import concourse.tile as tile
from concourse import bacc, mybir
from concourse.bass_utils import run_bass_kernel_spmd

B, T, N, H, E, KCH, HOR, OD = 64, 12, 1024, 64, 16, 3, 12, 1
NCORES, NL = 8, 128          # cores, nodes per core
FB = B * NL                  # 8192
FH = B * H                   # 4096
EPS = 1e-12
BF16 = mybir.dt.bfloat16
F32 = mybir.dt.float32
NPBF = ml_dtypes.bfloat16

_CACHE = {}


# --------------------------------------------------------------------------
# host precompute (weight-only constant folding + x-channel rows)
# --------------------------------------------------------------------------

def _host_precompute(inputs):
    f32 = np.float32
    node_emb = np.asarray(inputs["node_emb"], f32)
    time_emb = np.asarray(inputs["time_emb"], f32)
    src = np.asarray(inputs["source"], f32)[..., 0]            # [B,T,N]

    def ln(x, g, b):
        m = x.mean(-1, keepdims=True)
        v = x.var(-1, keepdims=True)
        return (x - m) / np.sqrt(v + EPS) * g + b

    def support(e):
        logits = e @ e.T
        s = np.exp(logits - logits.max(1, keepdims=True))
        s /= s.sum(1, keepdims=True)
        return s, 2.0 * s @ s - np.eye(N, dtype=f32)

    gate_wp = np.asarray(inputs["gate_wp"], f32)
    upd_wp = np.asarray(inputs["upd_wp"], f32)
    gate_bp = np.asarray(inputs["gate_bp"], f32)
    upd_bp = np.asarray(inputs["upd_bp"], f32)

    names = ["w1g", "w2g", "w1u", "w2u", "stg", "s2tg", "stu", "s2tu", "xr"]
    acc = {c: {k: [] for k in names} for c in range(NCORES)}

    for t in range(T):
        eg = ln(node_emb + time_emb[t][None, :],
                np.asarray(inputs["gate_lng"], f32),
                np.asarray(inputs["gate_lnb"], f32))
        eu = ln(node_emb + time_emb[t][None, :],
                np.asarray(inputs["upd_lng"], f32),
                np.asarray(inputs["upd_lnb"], f32))
        sg, s2g = support(eg)
        su, s2u = support(eu)
        wg = np.einsum("nd,dkio->nkio", eg, gate_wp)           # [N,3,65,128]
        wu = np.einsum("nd,dkio->nkio", eu, upd_wp)            # [N,3,65,64]
        bg = eg @ gate_bp                                      # [N,128]
        bu = eu @ upd_bp                                       # [N,64]
        xt = src[:, t, :]                                      # [B,N]
        xrows = np.stack([xt, xt @ sg.T, xt @ s2g.T,
                          xt, xt @ su.T, xt @ s2u.T], 0)       # [6,B,N]

        for c in range(NCORES):
            lo, hi = c * NL, (c + 1) * NL

            def pack_w(w, bias, O):
                mine = w[lo:hi]                                # [NL,3,65,O]
                w1 = np.concatenate([mine[:, 0, 1:, :], mine[:, 1, 1:, :]], 1)
                w2 = np.concatenate([mine[:, 2, 1:, :], mine[:, 0, 0:1, :],
                                     mine[:, 1, 0:1, :], mine[:, 2, 0:1, :],
                                     bias[lo:hi][:, None, :]], 1)
                # [NL, rows, O] -> [rows, NL*O]
                return (w1.transpose(1, 0, 2).reshape(128, NL * O),
                        w2.transpose(1, 0, 2).reshape(68, NL * O))

            w1g_, w2g_ = pack_w(wg, bg, 2 * H)
            w1u_, w2u_ = pack_w(wu, bu, H)
            acc[c]["w1g"].append(w1g_); acc[c]["w2g"].append(w2g_)
            acc[c]["w1u"].append(w1u_); acc[c]["w2u"].append(w2u_)

            def pack_s(s):
                # lhsT chunks: [128 (m within chunk), kc*128 + n_local]
                smt = s[lo:hi, :].T                            # [N(m), NL]
                return smt.reshape(8, 128, NL).transpose(1, 0, 2).reshape(128, 8 * NL)

            acc[c]["stg"].append(pack_s(sg)); acc[c]["s2tg"].append(pack_s(s2g))
            acc[c]["stu"].append(pack_s(su)); acc[c]["s2tu"].append(pack_s(s2u))
            acc[c]["xr"].append(xrows[:, :, lo:hi].reshape(6, FB))

    per_core = [dict() for _ in range(NCORES)]
    for c in range(NCORES):
        for k in names:
            per_core[c][k] = np.ascontiguousarray(np.stack(acc[c][k]), dtype=NPBF)

    # final-stage constants (same on every core)
    cw = np.asarray(inputs["conv_w"], f32)                     # [12,64]
    g = np.asarray(inputs["out_lng"], f32)
    be = np.asarray(inputs["out_lnb"], f32)
    cb = np.asarray(inputs["conv_b"], f32)
    A = cw * g[None, :]
    fa = np.zeros((64, 14), f32)
    fa[:, :12] = A.T
    fa[:, 12] = 1.0 / 64.0
    fc0 = (-A.sum(1))[None, :].astype(f32)          # [1,12]
    fcc = (cw @ be + cb)[:, None].astype(f32)
    ident = np.eye(128, dtype=NPBF)
    for c in range(NCORES):
        per_core[c]["fa"] = fa
        per_core[c]["fc0"] = fc0
        per_core[c]["fcc"] = fcc
        per_core[c]["ident"] = ident
    return per_core


# --------------------------------------------------------------------------
# device program (identical on all 8 cores; data differs)
# --------------------------------------------------------------------------

def _build_nc():
    nc = bacc.Bacc("TRN2", target_bir_lowering=False, debug=False,
                   num_devices=NCORES)

    def din(name, shape, dt=BF16):
        return nc.dram_tensor(name, shape, dt, kind="ExternalInput").ap()

    w1g_d = din("w1g", [T, 128, NL * 128])
    w2g_d = din("w2g", [T, 68, NL * 128])
    w1u_d = din("w1u", [T, 128, NL * 64])
    w2u_d = din("w2u", [T, 68, NL * 64])
    stg_d = din("stg", [T, 128, 1024])
    s2tg_d = din("s2tg", [T, 128, 1024])
    stu_d = din("stu", [T, 128, 1024])
    s2tu_d = din("s2tu", [T, 128, 1024])
    xr_d = din("xr", [T, 6, FB])
    fa_d = din("fa", [64, 14], F32)
    fc0_d = din("fc0", [1, 12], F32)
    fcc_d = din("fcc", [12, 1], F32)
    id_d = din("ident", [128, 128])
    out_d = nc.dram_tensor("out", [HOR, FB], F32, kind="ExternalOutput").ap()

    AF = mybir.ActivationFunctionType
    OP = mybir.AluOpType

    with tile.TileContext(nc) as tc:
        with (
            tc.tile_pool(name="persist", bufs=1) as pp,
            tc.tile_pool(name="wpool", bufs=1) as wp,
            tc.tile_pool(name="slices", bufs=3) as slp,
            tc.tile_pool(name="smats", bufs=1) as smp,
            tc.tile_pool(name="fin", bufs=1) as fin,
            tc.tile_pool(name="convps", bufs=3, space="PSUM") as convps,
            tc.tile_pool(name="pnps", bufs=2, space="PSUM") as pnps,
            tc.tile_pool(name="trps", bufs=2, space="PSUM") as trps,
            tc.tile_pool(name="dram", bufs=2, space="DRAM") as dram,
        ):
            # ---- persistent tiles ----
            Ht = pp.tile([64, FB], F32, tag="H")          # state, feat-major
            R1 = pp.tile([128, FB], BF16, tag="R1")       # [state^T|zs^T ; xg1^T]
            R2 = pp.tile([68, FB], BF16, tag="R2")        # [xg2^T ; 3 xr ; ones]
            ZR = pp.tile([128, FB], BF16, tag="ZR")       # sigmoid out (z ; r)
            HC = pp.tile([64, FB], BF16, tag="HC")        # tanh out
            XG1 = pp.tile([128, FH], BF16, tag="XG1")     # conv out (S) / AG shard
            XG2 = pp.tile([128, FH], BF16, tag="XG2")     # conv out (S2)
            IDT = pp.tile([128, 128], BF16, tag="IDT")

            nc.sync.dma_start(IDT[:], id_d[:])
            nc.vector.memset(Ht[:], 0.0)
            nc.gpsimd.memset(R2[0:64, :], 0.0)
            nc.gpsimd.memset(R2[64:68, :], 1.0)  # row 67 stays ones

            # strided node views: free = b*128+n -> [p, n, b]
            def nb(ap_):
                return ap_.rearrange("p (b n) -> p n b", n=NL)

            R1v, R2v = nb(R1[:]), nb(R2[:])
            ZRv, HCv = nb(ZR[:]), nb(HC[:])

            cp_v = lambda o, i: nc.vector.tensor_copy(o, i)
            cp_s = lambda o, i: nc.scalar.copy(o, i)
            ce = [cp_v, cp_s]                     # psum-capable copy fns

            def conv_phase(st_ap, s2t_ap, agout):
                """XG1/XG2 [128, FH] = (S^T).T @ state_all, (S2^T).T @ state."""
                stile = smp.tile([128, 1024], BF16, tag="stile")
                s2tile = smp.tile([128, 1024], BF16, tag="s2tile")
                nc.sync.dma_start(stile[:], st_ap)
                nc.sync.dma_start(s2tile[:], s2t_ap)
                for fc in range(8):
                    ps1 = convps.tile([128, 512], F32, tag="cps")
                    ps2 = convps.tile([128, 512], F32, tag="cps")
                    for kc in range(8):
                        sl = slp.tile([128, 512], BF16, tag="sl")
                        nc.sync.dma_start(
                            sl[:], agout[kc * 128:(kc + 1) * 128,
                                         fc * 512:(fc + 1) * 512])
                        nc.tensor.matmul(ps1[:], stile[:, kc * 128:(kc + 1) * 128],
                                         sl[:], start=(kc == 0), stop=(kc == 7))
                        nc.tensor.matmul(ps2[:], s2tile[:, kc * 128:(kc + 1) * 128],
                                         sl[:], start=(kc == 0), stop=(kc == 7))
                    ce[fc % 2](XG1[:, fc * 512:(fc + 1) * 512], ps1[:])
                    ce[(fc + 1) % 2](XG2[:, fc * 512:(fc + 1) * 512], ps2[:])

            def xg_transposes():
                """XG1 -> R1[64:128], XG2 -> R2[0:64]  (to feat-major)."""
                for b in range(B):
                    tp1 = trps.tile([128, 128], BF16, tag="tr")
                    nc.tensor.transpose(tp1[64:128, :], XG1[:, b * H:(b + 1) * H],
                                        IDT[:])
                    ce[b % 2](R1[64:128, b * NL:(b + 1) * NL], tp1[64:128, :])
                    tp2 = trps.tile([128, 128], BF16, tag="tr")
                    nc.tensor.transpose(tp2[0:64, :], XG2[:, b * H:(b + 1) * H],
                                        IDT[:])
                    ce[(b + 1) % 2](R2[0:64, b * NL:(b + 1) * NL], tp2[0:64, :])

            def allgather(src64):
                """src64 [64, FB] bf16 feat-major -> gathered [1024, FH] DRAM."""
                for b in range(B):
                    tp = trps.tile([128, 64], BF16, tag="tr")
                    nc.tensor.transpose(tp[:], src64[:, b * 128:(b + 1) * 128],
                                        IDT[0:64, 0:64])
                    ce[b % 2](XG1[:, b * H:(b + 1) * H], tp[:])
                agin = dram.tile([128, FH], BF16, tag="agin")
                agout = dram.tile([1024, FH], BF16, tag="agout")
                nc.sync.dma_start(agin[:], XG1[:])
                nc.gpsimd.collective_compute(
                    "AllGather", OP.bypass,
                    replica_groups=[list(range(NCORES))],
                    ins=[agin.opt()], outs=[agout.opt()])
                return agout

            def pernode(w1_t, w2_t, O, outv, func, first):
                """per-node matmuls; 8 nodes share one psum bank; fused act."""
                for g0 in range(0, NL, 8):
                    pg = pnps.tile([128, 512], F32, tag="pn")
                    for j in range(8):
                        n = g0 + j
                        o_sl = pg[0:O, j * B:(j + 1) * B]
                        if not first:
                            nc.tensor.matmul(
                                o_sl, w1_t[:, n * O:(n + 1) * O],
                                R1v[:, n:n + 1, :], start=True, stop=False)
                        nc.tensor.matmul(
                            o_sl, w2_t[:, n * O:(n + 1) * O],
                            R2v[0:68, n:n + 1, :], start=first, stop=True)
                    nc.scalar.activation(
                        outv[0:O, g0:g0 + 8, :],
                        pg[0:O, :].rearrange("p (j b) -> p j b", b=B), func)

            for t in range(T):
                first = (t == 0)
                w1g_t = None
                if not first:
                    w1g_t = wp.tile([128, NL * 128], BF16, tag="w1")
                    nc.sync.dma_start(w1g_t[:], w1g_d[t])
                w2g_t = wp.tile([68, NL * 128], BF16, tag="w2")
                nc.sync.dma_start(w2g_t[:], w2g_d[t])

                # ---------------- gate branch ----------------
                nc.sync.dma_start(R2[64:67, :], xr_d[t, 0:3])
                if not first:
                    nc.vector.tensor_copy(R1[0:64, :], Ht[:])   # state^T (bf16)
                    with nc.named_scope("ag"):
                        agout = allgather(R1[0:64, :])
                    with nc.named_scope("conv"):
                        conv_phase(stg_d[t], s2tg_d[t], agout)
                    with nc.named_scope("xgt"):
                        xg_transposes()
                with nc.named_scope("pernode"):
                    pernode(w1g_t, w2g_t, 128, ZRv, AF.Sigmoid, first)
                # zs = z * state  (bf16) -> R1 rows 0:64
                nc.vector.tensor_tensor(R1[0:64, :], ZR[0:64, :], Ht[:],
                                        op=OP.mult)

                # ---------------- update branch ----------------
                w1u_t = None
                if not first:
                    w1u_t = wp.tile([128, NL * 64], BF16, tag="w1")
                    nc.sync.dma_start(w1u_t[:], w1u_d[t])
                w2u_t = wp.tile([68, NL * 64], BF16, tag="w2")
                nc.sync.dma_start(w2u_t[:], w2u_d[t])
                nc.sync.dma_start(R2[64:67, :], xr_d[t, 3:6])
                if not first:
                    with nc.named_scope("ag"):
                        agout2 = allgather(R1[0:64, :])
                    with nc.named_scope("conv"):
                        conv_phase(stu_d[t], s2tu_d[t], agout2)
                    with nc.named_scope("xgt"):
                        xg_transposes()
                with nc.named_scope("pernode"):
                    pernode(w1u_t, w2u_t, 64, HCv, AF.Tanh, first)
                # bring r down to partitions 0:64 (z there is dead by now)
                nc.sync.dma_start(ZR[0:64, :], ZR[64:128, :])
                # h = r*h + (1-r)*hc,  in place:
                nc.vector.tensor_tensor(Ht[:], Ht[:], ZR[0:64, :], op=OP.mult)
                nc.gpsimd.tensor_scalar(ZR[0:64, :], ZR[0:64, :], -1.0, 1.0,
                                        op0=OP.mult, op1=OP.add)   # 1-r
                nc.vector.tensor_tensor(HC[:], HC[:], ZR[0:64, :], op=OP.mult)
                nc.vector.tensor_tensor(Ht[:], Ht[:], HC[:], op=OP.add)

            # ---------------- final LN + end conv ----------------
            FA = pp.tile([64, 14], F32, tag="FA")
            FC0 = pp.tile([1, 12], F32, tag="FC0")
            ON12 = pp.tile([1, 12], F32, tag="ON12")
            FCC = pp.tile([12, 1], F32, tag="FCC")
            nc.sync.dma_start(FA[:], fa_d[:])
            nc.sync.dma_start(FC0[:], fc0_d[:])
            nc.vector.memset(ON12[:], 1.0)
            nc.sync.dma_start(FCC[:], fcc_d[:])

            for fc in range(16):
              with nc.named_scope("final"):
                sl_ = slice(fc * 512, (fc + 1) * 512)
                sq = fin.tile([64, 512], F32, tag="fsq")
                nc.scalar.activation(sq[:], Ht[:, sl_], AF.Square)
                psA = convps.tile([12, 512], F32, tag="cps")
                nc.tensor.matmul(psA[:], FA[:, 0:12], Ht[:, sl_],
                                 start=True, stop=True)
                psM = convps.tile([1, 512], F32, tag="cps")
                nc.tensor.matmul(psM[:], FA[:, 12:13], Ht[:, sl_],
                                 start=True, stop=True)
                psB = convps.tile([1, 512], F32, tag="cps")
                nc.tensor.matmul(psB[:], FA[:, 12:13], sq[:],
                                 start=True, stop=True)
                sA = fin.tile([12, 512], F32, tag="fsA")
                nc.vector.tensor_copy(sA[:], psA[:])
                sM = fin.tile([1, 512], F32, tag="fsM")
                nc.scalar.copy(sM[:], psM[:])
                sM2 = fin.tile([1, 512], F32, tag="fsM2")
                nc.scalar.copy(sM2[:], psB[:])
                v = fin.tile([1, 512], F32, tag="fv")
                nc.vector.tensor_tensor(v[:], sM[:], sM[:], op=OP.mult)
                nc.vector.tensor_tensor(v[:], sM2[:], v[:], op=OP.subtract)
                nc.vector.tensor_scalar_add(v[:], v[:], EPS)
                sd = fin.tile([1, 512], F32, tag="fsd")
                nc.scalar.activation(sd[:], v[:], AF.Sqrt)
                nc.vector.reciprocal(v[:], sd[:])          # v = rstd
                mr = fin.tile([1, 512], F32, tag="fmr")
                nc.vector.tensor_tensor(mr[:], sM[:], v[:], op=OP.mult)
                psR = convps.tile([12, 512], F32, tag="cps")
                nc.tensor.matmul(psR[:], ON12[:], v[:], start=True, stop=True)
                psM = convps.tile([12, 512], F32, tag="cps")
                nc.tensor.matmul(psM[:], FC0[:], mr[:], start=True, stop=True)
                och = fin.tile([12, 512], F32, tag="foch")
                nc.vector.tensor_tensor(och[:], sA[:], psR[:], op=OP.mult)
                nc.vector.tensor_tensor(och[:], och[:], psM[:], op=OP.add)
                nc.vector.tensor_scalar_add(och[:], och[:], FCC[:, 0:1])
                nc.sync.dma_start(out_d[:, sl_], och[:])

    nc.compile()
    return nc


# --------------------------------------------------------------------------
# entry point
# --------------------------------------------------------------------------

def kernel(**inputs) -> np.ndarray:
    per_core = _host_precompute(inputs)
    if "nc" not in _CACHE:
        _CACHE["nc"] = _build_nc()
    res = run_bass_kernel_spmd(_CACHE["nc"], per_core, list(range(NCORES)))
    full = np.zeros((B, HOR, N, OD), np.float32)
    for c in range(NCORES):
        co = np.asarray(res.results[c]["out"], np.float32).reshape(HOR, B, NL)
        full[:, :, c * NL:(c + 1) * NL, 0] = co.transpose(1, 0, 2)
    return full



# revision 18
# speedup vs baseline: 1.4137x; 1.4137x over previous
"""DAAGCN Trainium2 kernel — node-sharded across 8 NeuronCores (V2).

Strategy: shard the 1024 graph nodes over 8 cores (128 each), full batch
B=64 per core. Supports S_t, Chebyshev S2_t, per-node weights and the
x-channel rows are constant-folded on the host. Each GRU step on device:
chunked AllGather(state, node-major) pipelined with a feat-major direct
graph conv (stationary = gathered state chunks, moving = S^T tiles), so
the conv output lands directly in feature-major layout (no transpose
phase). Per-node matmuls stream w1 by node-group; activations write
contiguous [O, n*64+b] slices. Final LayerNorm+end_conv folded into
matmuls with column-statistics rows.

Layout conventions (per core):
  feat-major: [feat, n_local*64 + b]   (free size FB = 8192)
  node-major: [n,   b*64 + i]          (free size FH = 4096)
"""
import sys

sys.path.insert(0, "/opt/trn_rl_repo")

import numpy as np
import ml_dtypes

import concourse.bass as bass
import concourse.tile as tile
from concourse import bacc, mybir
from concourse.bass_utils import run_bass_kernel_spmd

B, T, N, H, E, KCH, HOR, OD = 64, 12, 1024, 64, 16, 3, 12, 1
NCORES, NL = 8, 128          # cores, nodes per core
FB = NL * B                  # 8192  (free = n*64 + b)
FH = B * H                   # 4096  (free = b*64 + i)
CH = 4                       # AllGather chunks per branch
CB = B // CH                 # 16 b's per chunk
EPS = 1e-12
BF16 = mybir.dt.bfloat16
F32 = mybir.dt.float32
NPBF = ml_dtypes.bfloat16

_CACHE = {}


# --------------------------------------------------------------------------
# host precompute (weight-only constant folding + x-channel rows)
# --------------------------------------------------------------------------

def _host_precompute(inputs):
    f32 = np.float32
    node_emb = np.asarray(inputs["node_emb"], f32)
    time_emb = np.asarray(inputs["time_emb"], f32)
    src = np.asarray(inputs["source"], f32)[..., 0]            # [B,T,N]

    def ln(x, g, b):
        m = x.mean(-1, keepdims=True)
        v = x.var(-1, keepdims=True)
        return (x - m) / np.sqrt(v + EPS) * g + b

    def support(e):
        logits = e @ e.T
        s = np.exp(logits - logits.max(1, keepdims=True))
        s /= s.sum(1, keepdims=True)
        return s, 2.0 * s @ s - np.eye(N, dtype=f32)

    gate_wp = np.asarray(inputs["gate_wp"], f32)
    upd_wp = np.asarray(inputs["upd_wp"], f32)
    gate_bp = np.asarray(inputs["gate_bp"], f32)
    upd_bp = np.asarray(inputs["upd_bp"], f32)

    names = ["w1g", "w2g", "w1u", "w2u", "stg", "s2tg", "stu", "s2tu", "xr"]
    acc = {c: {k: [] for k in names} for c in range(NCORES)}

    for t in range(T):
        eg = ln(node_emb + time_emb[t][None, :],
                np.asarray(inputs["gate_lng"], f32),
                np.asarray(inputs["gate_lnb"], f32))
        eu = ln(node_emb + time_emb[t][None, :],
                np.asarray(inputs["upd_lng"], f32),
                np.asarray(inputs["upd_lnb"], f32))
        sg, s2g = support(eg)
        su, s2u = support(eu)
        wg = np.einsum("nd,dkio->nkio", eg, gate_wp)           # [N,3,65,128]
        wu = np.einsum("nd,dkio->nkio", eu, upd_wp)            # [N,3,65,64]
        bg = eg @ gate_bp                                      # [N,128]
        bu = eu @ upd_bp                                       # [N,64]
        xt = src[:, t, :]                                      # [B,N]
        xrows = np.stack([xt, xt @ sg.T, xt @ s2g.T,
                          xt, xt @ su.T, xt @ s2u.T], 0)       # [6,B,N]

        for c in range(NCORES):
            lo, hi = c * NL, (c + 1) * NL

            def pack_w(w, bias, O):
                mine = w[lo:hi]                                # [NL,3,65,O]
                w1 = np.concatenate([mine[:, 0, 1:, :], mine[:, 1, 1:, :]], 1)
                w2 = np.concatenate([mine[:, 2, 1:, :], mine[:, 0, 0:1, :],
                                     mine[:, 1, 0:1, :], mine[:, 2, 0:1, :],
                                     bias[lo:hi][:, None, :]], 1)
                # [NL, rows, O] -> [rows, NL*O]
                return (w1.transpose(1, 0, 2).reshape(128, NL * O),
                        w2.transpose(1, 0, 2).reshape(68, NL * O))

            w1g_, w2g_ = pack_w(wg, bg, 2 * H)
            w1u_, w2u_ = pack_w(wu, bu, H)
            acc[c]["w1g"].append(w1g_); acc[c]["w2g"].append(w2g_)
            acc[c]["w1u"].append(w1u_); acc[c]["w2u"].append(w2u_)

            def pack_s(s):
                # rhs chunks: [128 (m within chunk), kc*128 + n_local]
                smt = s[lo:hi, :].T                            # [N(m), NL]
                return smt.reshape(8, 128, NL).transpose(1, 0, 2).reshape(128, 8 * NL)

            acc[c]["stg"].append(pack_s(sg)); acc[c]["s2tg"].append(pack_s(s2g))
            acc[c]["stu"].append(pack_s(su)); acc[c]["s2tu"].append(pack_s(s2u))
            # x rows in feat-major free layout n*64 + b
            acc[c]["xr"].append(
                np.ascontiguousarray(xrows[:, :, lo:hi].transpose(0, 2, 1))
                .reshape(6, FB))

    per_core = [dict() for _ in range(NCORES)]
    for c in range(NCORES):
        for k in names:
            per_core[c][k] = np.ascontiguousarray(np.stack(acc[c][k]), dtype=NPBF)

    # final-stage constants (same on every core)
    cw = np.asarray(inputs["conv_w"], f32)                     # [12,64]
    g = np.asarray(inputs["out_lng"], f32)
    be = np.asarray(inputs["out_lnb"], f32)
    cb = np.asarray(inputs["conv_b"], f32)
    A = cw * g[None, :]
    fa = np.zeros((64, 14), f32)
    fa[:, :12] = A.T
    fa[:, 12] = 1.0 / 64.0
    fc0 = (-A.sum(1))[None, :].astype(f32)          # [1,12]
    fcc = (cw @ be + cb)[:, None].astype(f32)
    ident = np.eye(128, dtype=NPBF)
    for c in range(NCORES):
        per_core[c]["fa"] = fa
        per_core[c]["fc0"] = fc0
        per_core[c]["fcc"] = fcc
        per_core[c]["ident"] = ident
    return per_core


# --------------------------------------------------------------------------
# device program (identical on all 8 cores; data differs)
# --------------------------------------------------------------------------

def _build_nc():
    nc = bacc.Bacc("TRN2", target_bir_lowering=False, debug=False,
                   num_devices=NCORES)

    def din(name, shape, dt=BF16):
        return nc.dram_tensor(name, shape, dt, kind="ExternalInput").ap()

    w1g_d = din("w1g", [T, 128, NL * 128])
    w2g_d = din("w2g", [T, 68, NL * 128])
    w1u_d = din("w1u", [T, 128, NL * 64])
    w2u_d = din("w2u", [T, 68, NL * 64])
    stg_d = din("stg", [T, 128, 1024])
    s2tg_d = din("s2tg", [T, 128, 1024])
    stu_d = din("stu", [T, 128, 1024])
    s2tu_d = din("s2tu", [T, 128, 1024])
    xr_d = din("xr", [T, 6, FB])
    fa_d = din("fa", [64, 14], F32)
    fc0_d = din("fc0", [1, 12], F32)
    fcc_d = din("fcc", [12, 1], F32)
    id_d = din("ident", [128, 128])
    out_d = nc.dram_tensor("out", [HOR, FB], F32, kind="ExternalOutput").ap()

    AF = mybir.ActivationFunctionType
    OP = mybir.AluOpType

    with tile.TileContext(nc) as tc:
        with (
            tc.tile_pool(name="persist", bufs=1) as pp,
            tc.tile_pool(name="w2pool", bufs=1) as w2p,
            tc.tile_pool(name="w1pool", bufs=6) as w1p,
            tc.tile_pool(name="spool", bufs=1) as sp,
            tc.tile_pool(name="hall", bufs=2) as hp,
            tc.tile_pool(name="fin", bufs=1) as fin,
            tc.tile_pool(name="convps", bufs=4, space="PSUM") as cvps,
            tc.tile_pool(name="pnps", bufs=2, space="PSUM") as pnps,
            tc.tile_pool(name="trps", bufs=2, space="PSUM") as trps,
            tc.tile_pool(name="dram", bufs=2, space="DRAM") as dram,
        ):
            # ---- persistent tiles ----
            Ht = pp.tile([64, FB], F32, tag="H")          # state, feat-major
            R1 = pp.tile([128, FB], BF16, tag="R1")       # [state^T|zs^T ; xg1^T]
            R2 = pp.tile([68, FB], BF16, tag="R2")        # [xg2^T ; 3 xr ; ones]
            ZR = pp.tile([128, FB], BF16, tag="ZR")       # sigmoid out (z ; r)
            HC = pp.tile([64, FB], BF16, tag="HC")        # tanh out
            AGI = pp.tile([128, FH], BF16, tag="AGI")     # node-major AG input
            IDT = pp.tile([128, 128], BF16, tag="IDT")

            nc.sync.dma_start(IDT[:], id_d[:])
            nc.vector.memset(Ht[:], 0.0)
            nc.gpsimd.memset(R2[0:64, :], 0.0)
            nc.gpsimd.memset(R2[64:68, :], 1.0)  # row 67 stays ones

            # feat-major strided views: free = n*64 + b -> [p, n, b]
            def nb(ap_):
                return ap_.rearrange("p (n b) -> p n b", b=B)

            R1v, R2v = nb(R1[:]), nb(R2[:])
            ZRv, HCv = nb(ZR[:]), nb(HC[:])

            cp_v = lambda o, i: nc.vector.tensor_copy(o, i)
            cp_s = lambda o, i: nc.scalar.copy(o, i)
            cp_g = lambda o, i: nc.gpsimd.tensor_copy(o, i)

            def allgather_chunks(src64, tagpfx):
                """src64 [64,FB] bf16 feat-major -> CH chunked AGs.

                Returns list of agout chunk tiles [1024, CB*64] (node-major)."""
                outs = []
                for c in range(CH):
                    with nc.named_scope("ag"):
                        for j in range(CB):
                            b = c * CB + j
                            tp = trps.tile([128, 64], BF16, tag="tr")
                            nc.tensor.transpose(tp[:], R1v[0:64, :, b],
                                                IDT[0:64, 0:64])
                            (cp_s if j % 2 else cp_v)(
                                AGI[:, b * H:(b + 1) * H], tp[:])
                        agin = dram.tile([128, CB * H], BF16,
                                         tag=f"agi{c}", name=f"agin{c}")
                        agout = dram.tile([1024, CB * H], BF16,
                                          tag=f"ago{c}", name=f"agout{c}",
                                          addr_space="Shared")
                        nc.sync.dma_start(
                            agin[:], AGI[:, c * CB * H:(c + 1) * CB * H])
                        nc.gpsimd.collective_compute(
                            "AllGather", OP.bypass,
                            replica_groups=[list(range(NCORES))],
                            ins=[agin.opt()], outs=[agout.opt()])
                        outs.append(agout)
                return outs

            def conv_phase(st_ap, s2t_ap, agouts, stag):
                """Feat-major conv: R1[64:128]=xg1^T, R2[0:64]=xg2^T."""
                stile = sp.tile([128, 1024], BF16, tag=f"st{stag}",
                                name="stile")
                s2tile = sp.tile([128, 1024], BF16, tag=f"s2{stag}",
                                 name="s2tile")
                nc.sync.dma_start(stile[:], st_ap)
                nc.sync.dma_start(s2tile[:], s2t_ap)
                ncp = 0
                for c in range(CH):
                    halls = []
                    for kc in range(8):
                        ht_ = hp.tile([128, CB * H], BF16, tag=f"h{kc}",
                                      name=f"hall{kc}")
                        nc.sync.dma_start(
                            ht_[:], agouts[c][kc * 128:(kc + 1) * 128, :])
                        halls.append(ht_)
                    for q in range(CB * H // 128):     # 8 bi-chunks of 128
                        ps1 = cvps.tile([128, 128], F32, tag="cps")
                        ps2 = cvps.tile([128, 128], F32, tag="cps")
                        for kc in range(8):
                            lhsT = halls[kc][:, q * 128:(q + 1) * 128]
                            nc.tensor.matmul(
                                ps1[:], lhsT, stile[:, kc * 128:(kc + 1) * 128],
                                start=(kc == 0), stop=(kc == 7))
                            nc.tensor.matmul(
                                ps2[:], lhsT, s2tile[:, kc * 128:(kc + 1) * 128],
                                start=(kc == 0), stop=(kc == 7))
                        qq = c * 8 + q
                        b0, b1 = 2 * qq, 2 * qq + 1
                        # psum rows 0:64 = b0 feats, 64:128 = b1 feats
                        cp_v(R1v[64:128, :, b0], ps1[0:64, :])     # cross
                        cp_s(R1v[64:128, :, b1], ps1[64:128, :])   # aligned
                        cp_s(R2v[0:64, :, b0], ps2[0:64, :])       # aligned
                        cp_v(R2v[0:64, :, b1], ps2[64:128, :])     # cross
                        ncp += 1

            def pernode(w1_d, w2_t, O, outv, func, first, t):
                """per-node matmuls; 8 nodes share one psum bank; fused act."""
                for g in range(16):
                    w1_t = None
                    if not first:
                        w1_t = w1p.tile([128, 8 * O], BF16, tag="w1",
                                        name="w1t")
                        nc.sync.dma_start(
                            w1_t[:], w1_d[t, :, g * 8 * O:(g + 1) * 8 * O])
                    pg = pnps.tile([128, 512], F32, tag="pn")
                    for j in range(8):
                        n = g * 8 + j
                        o_sl = pg[0:O, j * B:(j + 1) * B]
                        if not first:
                            nc.tensor.matmul(
                                o_sl, w1_t[:, j * O:(j + 1) * O],
                                R1v[:, n, :], start=True, stop=False)
                        nc.tensor.matmul(
                            o_sl, w2_t[:, n * O:(n + 1) * O],
                            R2v[0:68, n, :], start=first, stop=True)
                    nc.scalar.activation(
                        outv[0:O, g * 8:(g + 1) * 8, :], pg[0:O, :], func)

            for t in range(T):
                first = (t == 0)

                # ---------------- gate branch ----------------
                w2g_t = w2p.tile([68, NL * 128], BF16, tag="w2")
                nc.sync.dma_start(w2g_t[:], w2g_d[t])
                nc.sync.dma_start(R2[64:67, :], xr_d[t, 0:3])
                if not first:
                    nc.vector.tensor_copy(R1[0:64, :], Ht[:])   # cast to bf16
                    agouts = allgather_chunks(R1[0:64, :], "g")
                    with nc.named_scope("conv"):
                        conv_phase(stg_d[t], s2tg_d[t], agouts, "g")
                with nc.named_scope("pernode"):
                    pernode(w1g_d, w2g_t, 128, ZRv, AF.Sigmoid, first, t)
                # zs = z * state  (bf16) -> R1 rows 0:64
                with nc.named_scope("elem"):
                    nc.vector.tensor_tensor(R1[0:64, :], ZR[0:64, :], Ht[:],
                                            op=OP.mult)

                # ---------------- update branch ----------------
                w2u_t = w2p.tile([68, NL * 64], BF16, tag="w2")
                nc.sync.dma_start(w2u_t[:], w2u_d[t])
                nc.sync.dma_start(R2[64:67, :], xr_d[t, 3:6])
                if not first:
                    agouts2 = allgather_chunks(R1[0:64, :], "u")
                    with nc.named_scope("conv"):
                        conv_phase(stu_d[t], s2tu_d[t], agouts2, "u")
                with nc.named_scope("pernode"):
                    pernode(w1u_d, w2u_t, 64, HCv, AF.Tanh, first, t)
                with nc.named_scope("elem"):
                    # bring r down to partitions 0:64 (z there is dead now)
                    nc.vector.tensor_copy(ZR[0:64, :], ZR[64:128, :])
                    # h = r*(h - hc) + hc, in place:
                    nc.vector.tensor_tensor(Ht[:], Ht[:], HC[:], op=OP.subtract)
                    nc.vector.tensor_tensor(Ht[:], Ht[:], ZR[0:64, :],
                                            op=OP.mult)
                    nc.vector.tensor_tensor(Ht[:], Ht[:], HC[:], op=OP.add)

            # ---------------- final LN + end conv ----------------
            FA = pp.tile([64, 14], F32, tag="FA")
            FC0 = pp.tile([1, 12], F32, tag="FC0")
            ON12 = pp.tile([1, 12], F32, tag="ON12")
            FCC = pp.tile([12, 1], F32, tag="FCC")
            nc.sync.dma_start(FA[:], fa_d[:])
            nc.sync.dma_start(FC0[:], fc0_d[:])
            nc.vector.memset(ON12[:], 1.0)
            nc.sync.dma_start(FCC[:], fcc_d[:])

            for fc in range(16):
              with nc.named_scope("final"):
                sl_ = slice(fc * 512, (fc + 1) * 512)
                sq = fin.tile([64, 512], F32, tag="fsq")
                nc.scalar.activation(sq[:], Ht[:, sl_], AF.Square)
                psA = cvps.tile([12, 512], F32, tag="cps")
                nc.tensor.matmul(psA[:], FA[:, 0:12], Ht[:, sl_],
                                 start=True, stop=True)
                psM = cvps.tile([1, 512], F32, tag="cps")
                nc.tensor.matmul(psM[:], FA[:, 12:13], Ht[:, sl_],
                                 start=True, stop=True)
                psB = cvps.tile([1, 512], F32, tag="cps")
                nc.tensor.matmul(psB[:], FA[:, 12:13], sq[:],
                                 start=True, stop=True)
                sA = fin.tile([12, 512], F32, tag="fsA")
                nc.vector.tensor_copy(sA[:], psA[:])
                sM = fin.tile([1, 512], F32, tag="fsM", name="sM")[:]
                sM2 = fin.tile([1, 512], F32, tag="fsM2", name="sM2")[:]
                v = fin.tile([1, 512], F32, tag="fv", name="v")[:]
                nc.scalar.copy(sM, psM[:])
                nc.scalar.copy(sM2, psB[:])
                nc.vector.tensor_tensor(v, sM, sM, op=OP.mult)
                nc.vector.tensor_tensor(v, sM2, v, op=OP.subtract)
                nc.vector.tensor_scalar_add(v, v, EPS)
                nc.scalar.activation(sM2, v, AF.Sqrt)      # reuse as sqrt(var)
                nc.vector.reciprocal(v, sM2)               # v = rstd
                mr = sM2                                   # reuse slot for mr
                nc.vector.tensor_tensor(mr, sM, v, op=OP.mult)
                psR = cvps.tile([12, 512], F32, tag="cps")
                nc.tensor.matmul(psR[:], ON12[:], v, start=True, stop=True)
                psM2 = cvps.tile([12, 512], F32, tag="cps")
                nc.tensor.matmul(psM2[:], FC0[:], mr, start=True, stop=True)
                och = fin.tile([12, 512], F32, tag="foch")
                nc.vector.tensor_tensor(och[:], sA[:], psR[:], op=OP.mult)
                nc.vector.tensor_tensor(och[:], och[:], psM2[:], op=OP.add)
                nc.vector.tensor_scalar_add(och[:], och[:], FCC[:, 0:1])
                nc.sync.dma_start(out_d[:, sl_], och[:])

    nc.compile()
    return nc


# --------------------------------------------------------------------------
# entry point
# --------------------------------------------------------------------------

def kernel(**inputs) -> np.ndarray:
    per_core = _host_precompute(inputs)
    if "nc" not in _CACHE:
        _CACHE["nc"] = _build_nc()
    res = run_bass_kernel_spmd(_CACHE["nc"], per_core, list(range(NCORES)))
    full = np.zeros((B, HOR, N, OD), np.float32)
    for c in range(NCORES):
        co = np.asarray(res.results[c]["out"], np.float32).reshape(HOR, NL, B)
        full[:, :, c * NL:(c + 1) * NL, 0] = co.transpose(2, 0, 1)
    return full
